# revision 11
# baseline (speedup 1.0000x reference)
"""GNN message-passing kernel for Trainium2 (Bass/Tile), 8-core SPMD.

Model (from the reference):
  h0 = relu(x @ W_in.T + b_in).T            # [500, B] -> vertices 0..500
  for l in 1..7:   agg = segment_sum(w_edge * h[edge_src]) ; h_l = relu(agg)
  out = h[out_verts].T @ W_out.T + b_out    # [B, 10]

Device strategy:
  - Data-parallel over batch: 8 cores x 256 columns each.
  - The sparse aggregation is a dense matmul agg = A_l @ h_lower with
    A_l built on the host. A is streamed in fp8(e4m3) and the matmuls
    run in DoubleRow perf mode (256-deep contraction per instruction at
    0.5 cyc/row), which is ~4x the bf16 MAC rate.
  - Precision: A is scaled by 16 and split into hi (+ lo residual on
    late layers); activations h are kept as a bf16 master copy plus an
    fp8 hi/lo pair. Per layer l the aggregation computes
        DR2:  A_hi @ (h_hi + h_lo)              (layers 1..5)
        DR3:  A_hi @ (h_hi + h_lo) + A_lo @ h_hi (layers 6..7)
    Late layers get the A residual because their error feeds the output
    head directly; early-layer errors wash out through the 32-edge
    averaging of subsequent layers.
  - Input layer and output head stay bf16.
  - Vertex space padded to 512/layer: every layer is 4 partition tiles
    of 128, and DoubleRow pairs two 128-tiles per instruction.
  - The out_verts gather is folded into a scattered W_out on the host.
"""

import sys

try:
    import concourse  # noqa: F401  (provided by the axon site-path)
except ImportError:
    sys.path.insert(0, "/opt/trn_rl_repo")

import numpy as np
from ml_dtypes import bfloat16, float8_e4m3

# ---- problem geometry (fixed by the problem spec) ----
B = 2048            # total batch
NC = 8              # cores
BL = B // NC        # 256 batch columns per core
IN_DIM = 784
K_IN = 896          # 784 padded to 7*128
PER = 500           # vertices per layer
PAD = 512           # padded vertices per layer (4*128)
L = 8               # layers (layer 0 = input layer)
NT = 4 * L          # 32 h tiles of 128 vertices
OUT_DIM = 10

A_SCALE = 16.0      # fp8 subnormal rescue; undone by the act scale=1/16
# layers carrying the A_lo residual correction (DR3); others are DR2
DR3_LAYERS = (6, 7)
# DMA chunk size for DR3 layers, in k-pairs (bounds the tail exposure)
DR3_CHUNK_KP = 2

_PROG = None
_PROG_KEY = None
_LAST_IN_MAPS = None  # kept for external profiling harnesses


def _build_program(used_tiles):
    from concourse import bacc, tile
    import concourse.mybir as mybir

    f32 = mybir.dt.float32
    bf16 = mybir.dt.bfloat16
    fp8 = mybir.dt.float8e4
    AF = mybir.ActivationFunctionType
    DR = mybir.MatmulPerfMode.DoubleRow

    n_used = len(used_tiles)
    inv_s = 1.0 / A_SCALE
    nc = bacc.Bacc(None, target_bir_lowering=False)

    xT_d = nc.dram_tensor("xT", [128, 7, BL], bf16, kind="ExternalInput")
    win_d = nc.dram_tensor("W_inT", [4, 128, 7, 128], bf16, kind="ExternalInput")
    bin_d = nc.dram_tensor("b_inP", [128, 4], f32, kind="ExternalInput")
    a2_ds = {}
    a3_ds = {}
    for l in range(1, L):
        if l in DR3_LAYERS:
            a3_ds[l] = nc.dram_tensor(
                f"A3_{l}", [128, 2 * l, 2, 2, PAD], fp8, kind="ExternalInput"
            )
        else:
            a2_ds[l] = nc.dram_tensor(
                f"A2_{l}", [128, 2 * l, 2, PAD], fp8, kind="ExternalInput"
            )
    wout_d = nc.dram_tensor(
        "W_outT", [128, n_used, OUT_DIM], bf16, kind="ExternalInput"
    )
    bout_d = nc.dram_tensor("b_outP", [OUT_DIM, 1], f32, kind="ExternalOutput"
                            if False else "ExternalInput")
    out_d = nc.dram_tensor("out", [OUT_DIM, BL], f32, kind="ExternalOutput")

    with tile.TileContext(nc) as tc:
        with (
            tc.tile_pool(name="const", bufs=1) as cpool,
            tc.tile_pool(name="hbuf", bufs=1) as hpool,
            tc.tile_pool(name="ps", bufs=7, space="PSUM") as ppool,
            tc.tile_pool(name="pso", bufs=1, space="PSUM") as opool,
            tc.tile_pool(name="outs", bufs=1) as spool,
        ):
            # ---- DMA issue: alternate SP/Act queues so per-instruction
            # setup (seq 565-667ns, HWDGE 625ns) pipelines; small tensors
            # ride mid-stream so the A stream starts as early as possible.
            dma_eng = [nc.sync, nc.scalar]
            dma_i = [0]

            def dma(dst, src):
                dma_eng[dma_i[0] % 2].dma_start(dst, src)
                dma_i[0] += 1

            win_tiles = [cpool.tile([128, 7, 128], bf16, name=f"win{m}")
                         for m in range(4)]
            xt_s = cpool.tile([128, 7, BL], bf16)
            dma(win_tiles[0][:], win_d[0])
            dma(xt_s[:, 0:2, :], xT_d[:, 0:2, :])
            dma(xt_s[:, 2:7, :], xT_d[:, 2:7, :])
            for m in range(1, 4):
                dma(win_tiles[m][:], win_d[m])
            bin_s = cpool.tile([128, 4], f32)
            wout_s = cpool.tile([128, n_used, OUT_DIM], bf16)
            bout_s = cpool.tile([OUT_DIM, 1], f32)

            # ---- A stream: whole-layer tiles for DR2, chunked for DR3 ----
            a2_s = {}
            a3_chunks = {}
            for l in range(1, L):
                if l not in DR3_LAYERS:
                    t = cpool.tile([128, 2 * l, 2, PAD], fp8, name=f"a2_{l}")
                    half = l  # split layer DMA in two for pipelining
                    dma(t[:, 0:half], a2_ds[l][:, 0:half])
                    dma(t[:, half:2 * l], a2_ds[l][:, half:2 * l])
                    a2_s[l] = t
                else:
                    chunks = []
                    nkp = 2 * l
                    for c0 in range(0, nkp, DR3_CHUNK_KP):
                        c1 = min(c0 + DR3_CHUNK_KP, nkp)
                        t = cpool.tile(
                            [128, c1 - c0, 2, 2, PAD], fp8, name=f"a3_{l}_{c0}"
                        )
                        dma(t[:], a3_ds[l][:, c0:c1])
                        chunks.append((c0, c1, t))
                    a3_chunks[l] = chunks
                if l == 1:
                    dma(bin_s[:], bin_d[:])
                elif l == 4:
                    dma(wout_s[:], wout_d[:])
                    dma(bout_s[:], bout_d[:])

            # ---- activation storage: one tile set per layer for precise
            # dependency tracking (a matmul reading layer j's h only waits
            # on layer j's activation writes, not the latest layer's) ----
            h_bf_t = [hpool.tile([128, 4, BL], bf16, name=f"hbf{j}")
                      for j in range(L)]
            h_hi_t = [hpool.tile([128, 4, BL], fp8, name=f"hhi{j}")
                      for j in range(L)]
            h_lo_t = [hpool.tile([128, 4, BL], fp8, name=f"hlo{j}")
                      for j in range(L)]
            zeros_s = cpool.tile([128, BL], f32)
            nc.vector.memset(zeros_s[:], 0.0)

            def rhs_slice(arr_t, kp):
                j, p = kp // 2, kp % 2
                return arr_t[j][:, 2 * p:2 * p + 2, :]

            def stt_relu(eng, out, ps, scale, bias):
                if isinstance(bias, float):
                    eng.scalar_tensor_tensor(
                        out, ps[:], scale, zeros_s[:],
                        mybir.AluOpType.mult, mybir.AluOpType.max,
                    )
                else:
                    eng.scalar_tensor_tensor(
                        out, ps[:], bias, zeros_s[:],
                        mybir.AluOpType.add, mybir.AluOpType.max,
                    )

            def split_layer(j, psums, scale, biases):
                """4 psums -> h_hi (Act m0,2 / DVE m1,3), h_bf, h_lo.

                h_hi lands fast (parity-split across two engines) since the
                next layer's first matmuls need it in m order; h_bf next
                (frees psums); h_lo last (only needed at the end of the
                next layer).
                """
                for m in (0, 2):
                    nc.scalar.activation(
                        h_hi_t[j][:, m, :], psums[m][:], AF.Relu,
                        bias=biases[m], scale=scale,
                    )
                for m in (1, 3):
                    stt_relu(nc.vector, h_hi_t[j][:, m, :], psums[m],
                             scale, biases[m])
                for m in (1, 3):
                    nc.scalar.activation(
                        h_bf_t[j][:, m, :], psums[m][:], AF.Relu,
                        bias=biases[m], scale=scale,
                    )
                for m in (0, 2):
                    stt_relu(nc.vector, h_bf_t[j][:, m, :], psums[m],
                             scale, biases[m])
                for m in range(4):
                    nc.vector.tensor_sub(
                        h_lo_t[j][:, m, :], h_bf_t[j][:, m, :],
                        h_hi_t[j][:, m, :]
                    )

            def bf_only(j, m, ps, scale, bias):
                """Last layer: only h_bf is consumed (by the head)."""
                if m in (0, 2):
                    nc.scalar.activation(
                        h_bf_t[j][:, m, :], ps[:], AF.Relu,
                        bias=bias, scale=scale,
                    )
                else:
                    stt_relu(nc.vector, h_bf_t[j][:, m, :], ps, scale, bias)

            # head bookkeeping: emit used-tile matmuls as soon as the
            # owning layer's h_bf lands
            used_by_layer = {}
            for i, kt in enumerate(used_tiles):
                used_by_layer.setdefault(kt // 4, []).append((i, kt))
            pso = opool.tile([OUT_DIM, BL], f32)
            head_emitted = [0]

            def emit_head(j, ms=None):
                for i, kt in used_by_layer.get(j, []):
                    if ms is not None and (kt % 4) not in ms:
                        continue
                    nc.tensor.matmul(
                        pso[:],
                        wout_s[:, i, :],
                        h_bf_t[j][:, kt % 4, :],
                        start=(head_emitted[0] == 0),
                        stop=(head_emitted[0] == len(used_tiles) - 1),
                    )
                    head_emitted[0] += 1

            # ---- input layer: h[0:4] = relu(W_in.T.T @ xT + b_in) ----
            pins = [ppool.tile([128, BL], f32, tag="ps", name=f"pin{m}")
                    for m in range(4)]
            for m in range(4):
                for kt in range(7):
                    nc.tensor.matmul(
                        pins[m][:],
                        win_tiles[m][:, kt, :],
                        xt_s[:, kt, :],
                        start=(kt == 0),
                        stop=(kt == 6),
                    )
            split_layer(0, pins, 1.0, [bin_s[:, m:m + 1] for m in range(4)])
            emit_head(0)

            # ---- hidden layers (fp8 DoubleRow) ----
            for l in range(1, L):
                nkp = 2 * l
                pls = [
                    ppool.tile([128, BL], f32, tag="ps", name=f"pl{l}_{m}")
                    for m in range(4)
                ]
                n_passes = 3 if l in DR3_LAYERS else 2
                per_m = nkp * n_passes  # accumulation-group length per psum
                idx = 0  # index within each m's group (same for all m)
                # h_hi passes stream with the A chunks; the h_lo pass runs
                # last so the DVE has the whole layer to produce h_lo of the
                # just-finished previous layer.
                if l in DR3_LAYERS:
                    for c0, c1, t in a3_chunks[l]:
                        for kp in range(c0, c1):
                            ci = kp - c0
                            rhs_hi = rhs_slice(h_hi_t, kp)
                            for w_sel in (0, 1):
                                for m in range(4):
                                    nc.tensor.matmul(
                                        pls[m][:],
                                        t[:, ci, w_sel, :, m * 128:(m + 1) * 128],
                                        rhs_hi,
                                        start=(idx == 0),
                                        stop=(idx == per_m - 1),
                                        perf_mode=DR,
                                    )
                                idx += 1
                    for c0, c1, t in a3_chunks[l]:
                        for kp in range(c0, c1):
                            ci = kp - c0
                            rhs_lo = rhs_slice(h_lo_t, kp)
                            for m in range(4):
                                nc.tensor.matmul(
                                    pls[m][:],
                                    t[:, ci, 0, :, m * 128:(m + 1) * 128],
                                    rhs_lo,
                                    start=(idx == 0),
                                    stop=(idx == per_m - 1),
                                    perf_mode=DR,
                                )
                            idx += 1
                else:
                    t = a2_s[l]
                    for kp in range(nkp):
                        rhs_hi = rhs_slice(h_hi_t, kp)
                        for m in range(4):
                            nc.tensor.matmul(
                                pls[m][:],
                                t[:, kp, :, m * 128:(m + 1) * 128],
                                rhs_hi,
                                start=(idx == 0),
                                stop=(idx == per_m - 1),
                                perf_mode=DR,
                            )
                        idx += 1
                    for kp in range(nkp):
                        rhs_lo = rhs_slice(h_lo_t, kp)
                        for m in range(4):
                            nc.tensor.matmul(
                                pls[m][:],
                                t[:, kp, :, m * 128:(m + 1) * 128],
                                rhs_lo,
                                start=(idx == 0),
                                stop=(idx == per_m - 1),
                                perf_mode=DR,
                            )
                        idx += 1
                if l == L - 1:
                    for m in range(4):
                        bf_only(l, m, pls[m], inv_s, 0.0)
                else:
                    split_layer(l, pls, inv_s, [0.0] * 4)
                emit_head(l)

            # ---- output head epilogue ----
            out_s = spool.tile([OUT_DIM, BL], f32)
            nc.scalar.activation(out_s[:], pso[:], AF.Identity, bias=bout_s[:])
            nc.sync.dma_start(out_d[:], out_s[:])

    nc.compile()
    return nc


def _pack_ptiles(arr2d, n_tiles):
    """[n_tiles*128, F] row-major -> [128, n_tiles, F] partition-major."""
    f = arr2d.shape[1]
    return np.ascontiguousarray(
        arr2d.reshape(n_tiles, 128, f).transpose(1, 0, 2)
    )


def kernel(**inputs):
    x = np.asarray(inputs["x"], np.float32)
    W_in = np.asarray(inputs["W_in"], np.float32)
    b_in = np.asarray(inputs["b_in"], np.float32)
    w_edge = np.asarray(inputs["w_edge"], np.float32)
    W_out = np.asarray(inputs["W_out"], np.float32)
    b_out = np.asarray(inputs["b_out"], np.float32)
    edge_src = np.asarray(inputs["edge_src"]).astype(np.int64)
    edge_dst = np.asarray(inputs["edge_dst_local"]).astype(np.int64)
    offsets = np.asarray(inputs["edge_offsets"]).astype(np.int64)
    out_verts = np.asarray(inputs["out_verts"]).astype(np.int64)

    # ---- host-side packing ----
    shared = {}
    for l in range(1, L):
        s, e = int(offsets[l - 1]), int(offsets[l])
        At = np.zeros((l * PAD, PAD), np.float32)  # [src_padded, tgt]
        rows = (edge_src[s:e] // PER) * PAD + (edge_src[s:e] % PER)
        np.add.at(At, (rows, edge_dst[s:e]), w_edge[s:e])
        At *= A_SCALE
        A_hi = At.astype(float8_e4m3)
        A_lo = (At - A_hi.astype(np.float32)).astype(float8_e4m3)
        # [4l*128, 512] -> [2l kp, 2 kt, 128, 512] -> [128, 2l, 2, 512]
        def kp_form(a8):
            return np.ascontiguousarray(
                a8.reshape(2 * l, 2, 128, PAD).transpose(2, 0, 1, 3)
            )
        hi = kp_form(A_hi)
        if l in DR3_LAYERS:
            lo = kp_form(A_lo)
            # [128, 2l, 2(hilo), 2, 512]
            shared[f"A3_{l}"] = np.ascontiguousarray(
                np.stack([hi, lo], axis=2)
            )
        else:
            shared[f"A2_{l}"] = hi

    winT = np.zeros((K_IN, PAD), np.float32)
    winT[:IN_DIM, :PER] = W_in.T
    winT_re = np.ascontiguousarray(
        _pack_ptiles(winT, 7).reshape(128, 7, 4, 128).transpose(2, 0, 1, 3)
    ).astype(bfloat16)

    binP = np.zeros((PAD,), np.float32)
    binP[:PER] = b_in
    binP_re = np.ascontiguousarray(binP.reshape(4, 128).T)

    woutT = np.zeros((NT * 128, OUT_DIM), np.float32)
    pad_idx = (out_verts // PER) * PAD + (out_verts % PER)
    woutT[pad_idx, :] = W_out.T
    used_tiles = tuple(sorted(set(int(t) for t in pad_idx // 128)))
    woutT_re = np.ascontiguousarray(
        _pack_ptiles(woutT, NT)[:, list(used_tiles), :]
    ).astype(bfloat16)

    boutP = np.ascontiguousarray(b_out.reshape(OUT_DIM, 1))

    shared.update({
        "W_inT": winT_re,
        "b_inP": binP_re,
        "W_outT": woutT_re,
        "b_outP": boutP,
    })
    in_maps = []
    for c in range(NC):
        xT = np.zeros((K_IN, BL), np.float32)
        xT[:IN_DIM, :] = x[c * BL:(c + 1) * BL, :].T
        in_maps.append({"xT": _pack_ptiles(xT, 7).astype(bfloat16), **shared})

    from concourse.bass_utils import run_bass_kernel_spmd

    global _LAST_IN_MAPS, _PROG, _PROG_KEY
    _LAST_IN_MAPS = in_maps
    if _PROG is None or _PROG_KEY != used_tiles:
        _PROG = _build_program(used_tiles)
        _PROG_KEY = used_tiles
    res = run_bass_kernel_spmd(_PROG, in_maps, list(range(NC)))
    out = np.concatenate(
        [np.asarray(res.results[c]["out"], np.float32).T for c in range(NC)], axis=0
    )
    return np.ascontiguousarray(out)


# revision 12
# speedup vs baseline: 1.3392x; 1.3392x over previous
"""GNN message-passing kernel for Trainium2 (Bass/Tile), 8-core SPMD.

Model (from the reference):
  h0 = relu(x @ W_in.T + b_in).T            # [500, B] -> vertices 0..500
  for l in 1..7:   agg = segment_sum(w_edge * h[edge_src]) ; h_l = relu(agg)
  out = h[out_verts].T @ W_out.T + b_out    # [B, 10]

Device strategy:
  - Data-parallel over batch: 8 cores x 256 columns each.
  - The sparse aggregation is a dense matmul agg = A_l @ h_lower with
    A_l built on the host. A is streamed in fp8(e4m3) and the matmuls
    run in DoubleRow perf mode (256-deep contraction per instruction at
    0.5 cyc/row), which is ~4x the bf16 MAC rate.
  - Precision: A is scaled by 16 and split into hi (+ lo residual on
    late layers); activations h are kept as a bf16 master copy plus an
    fp8 hi/lo pair. Per layer l the aggregation computes
        DR2:  A_hi @ (h_hi + h_lo)              (layers 1..5)
        DR3:  A_hi @ (h_hi + h_lo) + A_lo @ h_hi (layers 6..7)
    Late layers get the A residual because their error feeds the output
    head directly; early-layer errors wash out through the 32-edge
    averaging of subsequent layers.
  - Input layer and output head stay bf16.
  - Vertex space padded to 512/layer: every layer is 4 partition tiles
    of 128, and DoubleRow pairs two 128-tiles per instruction.
  - The out_verts gather is folded into a scattered W_out on the host.
"""

import sys

try:
    import concourse  # noqa: F401  (provided by the axon site-path)
except ImportError:
    sys.path.insert(0, "/opt/trn_rl_repo")

import numpy as np
from ml_dtypes import bfloat16, float8_e4m3

# ---- problem geometry (fixed by the problem spec) ----
B = 2048            # total batch
NC = 8              # cores
BL = B // NC        # 256 batch columns per core
IN_DIM = 784
K_IN = 896          # 784 padded to 7*128
PER = 500           # vertices per layer
PAD = 512           # padded vertices per layer (4*128)
L = 8               # layers (layer 0 = input layer)
NT = 4 * L          # 32 h tiles of 128 vertices
OUT_DIM = 10

A_SCALE = 16.0      # fp8 subnormal rescue; undone by the act scale=1/16
# layers carrying the A_lo residual correction (DR3); others are DR2
DR3_LAYERS = (6, 7)
# DMA chunk size for DR3 layers, in k-pairs (bounds the tail exposure)
DR3_CHUNK_KP = 2

_PROG = None
_PROG_KEY = None
_LAST_IN_MAPS = None  # kept for external profiling harnesses


def _build_program(used_tiles):
    from concourse import bacc, tile
    import concourse.mybir as mybir

    f32 = mybir.dt.float32
    bf16 = mybir.dt.bfloat16
    fp8 = mybir.dt.float8e4
    AF = mybir.ActivationFunctionType
    DR = mybir.MatmulPerfMode.DoubleRow

    n_used = len(used_tiles)
    inv_s = 1.0 / A_SCALE
    nc = bacc.Bacc(None, target_bir_lowering=False)

    xT_d = nc.dram_tensor("xT", [128, 7, BL], bf16, kind="ExternalInput")
    win_d = nc.dram_tensor("W_inT", [4, 128, 7, 128], bf16, kind="ExternalInput")
    bin_d = nc.dram_tensor("b_inP", [128, 4], f32, kind="ExternalInput")
    a2_ds = {}
    a3_ds = {}
    for l in range(1, L):
        if l in DR3_LAYERS:
            a3_ds[l] = nc.dram_tensor(
                f"A3_{l}", [128, 2 * l, 2, 2, PAD], fp8, kind="ExternalInput"
            )
        else:
            a2_ds[l] = nc.dram_tensor(
                f"A2_{l}", [128, 2 * l, 2, PAD], fp8, kind="ExternalInput"
            )
    wout_d = nc.dram_tensor(
        "W_outT", [128, n_used, OUT_DIM], bf16, kind="ExternalInput"
    )
    bout_d = nc.dram_tensor("b_outP", [OUT_DIM, 1], f32, kind="ExternalOutput"
                            if False else "ExternalInput")
    out_d = nc.dram_tensor("out", [OUT_DIM, BL], f32, kind="ExternalOutput")

    with tile.TileContext(nc) as tc:
        with (
            tc.tile_pool(name="const", bufs=1) as cpool,
            tc.tile_pool(name="hbuf", bufs=1) as hpool,
            tc.tile_pool(name="ps", bufs=7, space="PSUM") as ppool,
            tc.tile_pool(name="pso", bufs=1, space="PSUM") as opool,
            tc.tile_pool(name="outs", bufs=1) as spool,
        ):
            # ---- DMA issue: alternate SP/Act queues so per-instruction
            # setup (seq 565-667ns, HWDGE 625ns) pipelines; small tensors
            # ride mid-stream so the A stream starts as early as possible.
            def dma(dst, src):
                nc.sync.dma_start(dst, src)

            win_tiles = [cpool.tile([128, 7, 128], bf16, name=f"win{m}")
                         for m in range(4)]
            xt_s = cpool.tile([128, 7, BL], bf16)
            dma(win_tiles[0][:], win_d[0])
            dma(xt_s[:, 0:2, :], xT_d[:, 0:2, :])
            dma(xt_s[:, 2:7, :], xT_d[:, 2:7, :])
            for m in range(1, 4):
                dma(win_tiles[m][:], win_d[m])
            bin_s = cpool.tile([128, 4], f32)
            wout_s = cpool.tile([128, n_used, OUT_DIM], bf16)
            bout_s = cpool.tile([OUT_DIM, 1], f32)

            # ---- A stream: whole-layer tiles for DR2, chunked for DR3 ----
            a2_s = {}
            a3_chunks = {}
            for l in range(1, L):
                if l not in DR3_LAYERS:
                    t = cpool.tile([128, 2 * l, 2, PAD], fp8, name=f"a2_{l}")
                    half = l  # split layer DMA in two for pipelining
                    dma(t[:, 0:half], a2_ds[l][:, 0:half])
                    dma(t[:, half:2 * l], a2_ds[l][:, half:2 * l])
                    a2_s[l] = t
                else:
                    chunks = []
                    nkp = 2 * l
                    for c0 in range(0, nkp, DR3_CHUNK_KP):
                        c1 = min(c0 + DR3_CHUNK_KP, nkp)
                        t = cpool.tile(
                            [128, c1 - c0, 2, 2, PAD], fp8, name=f"a3_{l}_{c0}"
                        )
                        dma(t[:], a3_ds[l][:, c0:c1])
                        chunks.append((c0, c1, t))
                    a3_chunks[l] = chunks
                if l == 1:
                    dma(bin_s[:], bin_d[:])
                elif l == 4:
                    dma(wout_s[:], wout_d[:])
                    dma(bout_s[:], bout_d[:])

            # ---- activation storage: one tile set per layer for precise
            # dependency tracking (a matmul reading layer j's h only waits
            # on layer j's activation writes, not the latest layer's) ----
            h_bf_t = [hpool.tile([128, 4, BL], bf16, name=f"hbf{j}")
                      for j in range(L)]
            h_hi_t = [hpool.tile([128, 4, BL], fp8, name=f"hhi{j}")
                      for j in range(L)]
            h_lo_t = [hpool.tile([128, 4, BL], fp8, name=f"hlo{j}")
                      for j in range(L)]
            zeros_s = cpool.tile([128, BL], f32)
            nc.vector.memset(zeros_s[:], 0.0)

            def rhs_slice(arr_t, kp):
                j, p = kp // 2, kp % 2
                return arr_t[j][:, 2 * p:2 * p + 2, :]

            def stt_relu(eng, out, ps, scale, bias):
                if isinstance(bias, float):
                    eng.scalar_tensor_tensor(
                        out, ps[:], scale, zeros_s[:],
                        mybir.AluOpType.mult, mybir.AluOpType.max,
                    )
                else:
                    eng.scalar_tensor_tensor(
                        out, ps[:], bias, zeros_s[:],
                        mybir.AluOpType.add, mybir.AluOpType.max,
                    )

            def split_layer(j, psums, scale, biases):
                """4 psums -> h_hi (Act m0,2 / DVE m1,3), h_bf, h_lo.

                h_hi lands fast (parity-split across two engines) since the
                next layer's first matmuls need it in m order; h_bf next
                (frees psums); h_lo last (only needed at the end of the
                next layer).
                """
                for m in (0, 2):
                    nc.scalar.activation(
                        h_hi_t[j][:, m, :], psums[m][:], AF.Relu,
                        bias=biases[m], scale=scale,
                    )
                for m in (1, 3):
                    stt_relu(nc.vector, h_hi_t[j][:, m, :], psums[m],
                             scale, biases[m])
                for m in (1, 3):
                    nc.scalar.activation(
                        h_bf_t[j][:, m, :], psums[m][:], AF.Relu,
                        bias=biases[m], scale=scale,
                    )
                for m in (0, 2):
                    stt_relu(nc.vector, h_bf_t[j][:, m, :], psums[m],
                             scale, biases[m])
                for m in range(4):
                    nc.vector.tensor_sub(
                        h_lo_t[j][:, m, :], h_bf_t[j][:, m, :],
                        h_hi_t[j][:, m, :]
                    )

            def bf_only(j, m, ps, scale, bias):
                """Last layer: only h_bf is consumed (by the head)."""
                if m in (0, 2):
                    nc.scalar.activation(
                        h_bf_t[j][:, m, :], ps[:], AF.Relu,
                        bias=bias, scale=scale,
                    )
                else:
                    stt_relu(nc.vector, h_bf_t[j][:, m, :], ps, scale, bias)

            # head bookkeeping: emit used-tile matmuls as soon as the
            # owning layer's h_bf lands
            used_by_layer = {}
            for i, kt in enumerate(used_tiles):
                used_by_layer.setdefault(kt // 4, []).append((i, kt))
            pso = opool.tile([OUT_DIM, BL], f32)
            head_emitted = [0]

            def emit_head(j, ms=None):
                for i, kt in used_by_layer.get(j, []):
                    if ms is not None and (kt % 4) not in ms:
                        continue
                    nc.tensor.matmul(
                        pso[:],
                        wout_s[:, i, :],
                        h_bf_t[j][:, kt % 4, :],
                        start=(head_emitted[0] == 0),
                        stop=(head_emitted[0] == len(used_tiles) - 1),
                    )
                    head_emitted[0] += 1

            # ---- input layer: h[0:4] = relu(W_in.T.T @ xT + b_in) ----
            pins = [ppool.tile([128, BL], f32, tag="ps", name=f"pin{m}")
                    for m in range(4)]
            for m in range(4):
                for kt in range(7):
                    nc.tensor.matmul(
                        pins[m][:],
                        win_tiles[m][:, kt, :],
                        xt_s[:, kt, :],
                        start=(kt == 0),
                        stop=(kt == 6),
                    )
            split_layer(0, pins, 1.0, [bin_s[:, m:m + 1] for m in range(4)])
            emit_head(0)

            # ---- hidden layers (fp8 DoubleRow) ----
            for l in range(1, L):
                nkp = 2 * l
                pls = [
                    ppool.tile([128, BL], f32, tag="ps", name=f"pl{l}_{m}")
                    for m in range(4)
                ]
                n_passes = 3 if l in DR3_LAYERS else 2
                per_m = nkp * n_passes  # accumulation-group length per psum
                idx = 0  # index within each m's group (same for all m)
                # h_hi passes stream with the A chunks; the h_lo pass runs
                # last so the DVE has the whole layer to produce h_lo of the
                # just-finished previous layer.
                if l in DR3_LAYERS:
                    for c0, c1, t in a3_chunks[l]:
                        for kp in range(c0, c1):
                            ci = kp - c0
                            rhs_hi = rhs_slice(h_hi_t, kp)
                            for w_sel in (0, 1):
                                for m in range(4):
                                    nc.tensor.matmul(
                                        pls[m][:],
                                        t[:, ci, w_sel, :, m * 128:(m + 1) * 128],
                                        rhs_hi,
                                        start=(idx == 0),
                                        stop=(idx == per_m - 1),
                                        perf_mode=DR,
                                    )
                                idx += 1
                    for c0, c1, t in a3_chunks[l]:
                        for kp in range(c0, c1):
                            ci = kp - c0
                            rhs_lo = rhs_slice(h_lo_t, kp)
                            for m in range(4):
                                nc.tensor.matmul(
                                    pls[m][:],
                                    t[:, ci, 0, :, m * 128:(m + 1) * 128],
                                    rhs_lo,
                                    start=(idx == 0),
                                    stop=(idx == per_m - 1),
                                    perf_mode=DR,
                                )
                            idx += 1
                else:
                    t = a2_s[l]
                    for kp in range(nkp):
                        rhs_hi = rhs_slice(h_hi_t, kp)
                        for m in range(4):
                            nc.tensor.matmul(
                                pls[m][:],
                                t[:, kp, :, m * 128:(m + 1) * 128],
                                rhs_hi,
                                start=(idx == 0),
                                stop=(idx == per_m - 1),
                                perf_mode=DR,
                            )
                        idx += 1
                    for kp in range(nkp):
                        rhs_lo = rhs_slice(h_lo_t, kp)
                        for m in range(4):
                            nc.tensor.matmul(
                                pls[m][:],
                                t[:, kp, :, m * 128:(m + 1) * 128],
                                rhs_lo,
                                start=(idx == 0),
                                stop=(idx == per_m - 1),
                                perf_mode=DR,
                            )
                        idx += 1
                if l == L - 1:
                    for m in range(4):
                        bf_only(l, m, pls[m], inv_s, 0.0)
                else:
                    split_layer(l, pls, inv_s, [0.0] * 4)
                emit_head(l)

            # ---- output head epilogue ----
            out_s = spool.tile([OUT_DIM, BL], f32)
            nc.scalar.activation(out_s[:], pso[:], AF.Identity, bias=bout_s[:])
            nc.sync.dma_start(out_d[:], out_s[:])

    nc.compile()
    return nc


def _pack_ptiles(arr2d, n_tiles):
    """[n_tiles*128, F] row-major -> [128, n_tiles, F] partition-major."""
    f = arr2d.shape[1]
    return np.ascontiguousarray(
        arr2d.reshape(n_tiles, 128, f).transpose(1, 0, 2)
    )


def kernel(**inputs):
    x = np.asarray(inputs["x"], np.float32)
    W_in = np.asarray(inputs["W_in"], np.float32)
    b_in = np.asarray(inputs["b_in"], np.float32)
    w_edge = np.asarray(inputs["w_edge"], np.float32)
    W_out = np.asarray(inputs["W_out"], np.float32)
    b_out = np.asarray(inputs["b_out"], np.float32)
    edge_src = np.asarray(inputs["edge_src"]).astype(np.int64)
    edge_dst = np.asarray(inputs["edge_dst_local"]).astype(np.int64)
    offsets = np.asarray(inputs["edge_offsets"]).astype(np.int64)
    out_verts = np.asarray(inputs["out_verts"]).astype(np.int64)

    # ---- host-side packing ----
    shared = {}
    for l in range(1, L):
        s, e = int(offsets[l - 1]), int(offsets[l])
        At = np.zeros((l * PAD, PAD), np.float32)  # [src_padded, tgt]
        rows = (edge_src[s:e] // PER) * PAD + (edge_src[s:e] % PER)
        np.add.at(At, (rows, edge_dst[s:e]), w_edge[s:e])
        At *= A_SCALE
        A_hi = At.astype(float8_e4m3)
        A_lo = (At - A_hi.astype(np.float32)).astype(float8_e4m3)
        # [4l*128, 512] -> [2l kp, 2 kt, 128, 512] -> [128, 2l, 2, 512]
        def kp_form(a8):
            return np.ascontiguousarray(
                a8.reshape(2 * l, 2, 128, PAD).transpose(2, 0, 1, 3)
            )
        hi = kp_form(A_hi)
        if l in DR3_LAYERS:
            lo = kp_form(A_lo)
            # [128, 2l, 2(hilo), 2, 512]
            shared[f"A3_{l}"] = np.ascontiguousarray(
                np.stack([hi, lo], axis=2)
            )
        else:
            shared[f"A2_{l}"] = hi

    winT = np.zeros((K_IN, PAD), np.float32)
    winT[:IN_DIM, :PER] = W_in.T
    winT_re = np.ascontiguousarray(
        _pack_ptiles(winT, 7).reshape(128, 7, 4, 128).transpose(2, 0, 1, 3)
    ).astype(bfloat16)

    binP = np.zeros((PAD,), np.float32)
    binP[:PER] = b_in
    binP_re = np.ascontiguousarray(binP.reshape(4, 128).T)

    woutT = np.zeros((NT * 128, OUT_DIM), np.float32)
    pad_idx = (out_verts // PER) * PAD + (out_verts % PER)
    woutT[pad_idx, :] = W_out.T
    used_tiles = tuple(sorted(set(int(t) for t in pad_idx // 128)))
    woutT_re = np.ascontiguousarray(
        _pack_ptiles(woutT, NT)[:, list(used_tiles), :]
    ).astype(bfloat16)

    boutP = np.ascontiguousarray(b_out.reshape(OUT_DIM, 1))

    shared.update({
        "W_inT": winT_re,
        "b_inP": binP_re,
        "W_outT": woutT_re,
        "b_outP": boutP,
    })
    in_maps = []
    for c in range(NC):
        xT = np.zeros((K_IN, BL), np.float32)
        xT[:IN_DIM, :] = x[c * BL:(c + 1) * BL, :].T
        in_maps.append({"xT": _pack_ptiles(xT, 7).astype(bfloat16), **shared})

    from concourse.bass_utils import run_bass_kernel_spmd

    global _LAST_IN_MAPS, _PROG, _PROG_KEY
    _LAST_IN_MAPS = in_maps
    if _PROG is None or _PROG_KEY != used_tiles:
        _PROG = _build_program(used_tiles)
        _PROG_KEY = used_tiles
    res = run_bass_kernel_spmd(_PROG, in_maps, list(range(NC)))
    out = np.concatenate(
        [np.asarray(res.results[c]["out"], np.float32).T for c in range(NC)], axis=0
    )
    return np.ascontiguousarray(out)


# revision 13
# speedup vs baseline: 1.3550x; 1.0118x over previous
"""GNN message-passing kernel for Trainium2 (Bass/Tile), 8-core SPMD.

Model (from the reference):
  h0 = relu(x @ W_in.T + b_in).T            # [500, B] -> vertices 0..500
  for l in 1..7:   agg = segment_sum(w_edge * h[edge_src]) ; h_l = relu(agg)
  out = h[out_verts].T @ W_out.T + b_out    # [B, 10]

Device strategy:
  - Data-parallel over batch: 8 cores x 256 columns each.
  - The sparse aggregation is a dense matmul agg = A_l @ h_lower with
    A_l built on the host. A is streamed in fp8(e4m3) and the matmuls
    run in DoubleRow perf mode (256-deep contraction per instruction at
    0.5 cyc/row), which is ~4x the bf16 MAC rate.
  - Precision: A is scaled by 16 and split into hi (+ lo residual on
    late layers); activations h are kept as a bf16 master copy plus an
    fp8 hi/lo pair. Per layer l the aggregation computes
        DR2:  A_hi @ (h_hi + h_lo)              (layers 1..5)
        DR3:  A_hi @ (h_hi + h_lo) + A_lo @ h_hi (layers 6..7)
    Late layers get the A residual because their error feeds the output
    head directly; early-layer errors wash out through the 32-edge
    averaging of subsequent layers.
  - Input layer and output head stay bf16.
  - Vertex space padded to 512/layer: every layer is 4 partition tiles
    of 128, and DoubleRow pairs two 128-tiles per instruction.
  - The out_verts gather is folded into a scattered W_out on the host.
"""

import sys

try:
    import concourse  # noqa: F401  (provided by the axon site-path)
except ImportError:
    sys.path.insert(0, "/opt/trn_rl_repo")

import numpy as np
from ml_dtypes import bfloat16, float8_e4m3

# ---- problem geometry (fixed by the problem spec) ----
B = 2048            # total batch
NC = 8              # cores
BL = B // NC        # 256 batch columns per core
IN_DIM = 784
K_IN = 896          # 784 padded to 7*128
PER = 500           # vertices per layer
PAD = 512           # padded vertices per layer (4*128)
L = 8               # layers (layer 0 = input layer)
NT = 4 * L          # 32 h tiles of 128 vertices
OUT_DIM = 10

A_SCALE = 16.0      # fp8 subnormal rescue; undone by the act scale=1/16
# layers carrying the A_lo residual correction (DR3); others are DR2
DR3_LAYERS = (6, 7)
# DMA chunk size for DR3 layers, in k-pairs (bounds the tail exposure)
DR3_CHUNK_KP = 2
WARMUP_MM = 34

_PROG = None
_PROG_KEY = None
_LAST_IN_MAPS = None  # kept for external profiling harnesses


def _build_program(used_tiles):
    from concourse import bacc, tile
    import concourse.mybir as mybir

    f32 = mybir.dt.float32
    bf16 = mybir.dt.bfloat16
    fp8 = mybir.dt.float8e4
    AF = mybir.ActivationFunctionType
    DR = mybir.MatmulPerfMode.DoubleRow

    n_used = len(used_tiles)
    inv_s = 1.0 / A_SCALE  # noqa - WARMUP_MM from module scope
    nc = bacc.Bacc(None, target_bir_lowering=False)

    xT_d = nc.dram_tensor("xT", [128, 7, BL], bf16, kind="ExternalInput")
    win_d = nc.dram_tensor("W_inT", [4, 128, 7, 128], bf16, kind="ExternalInput")
    bin_d = nc.dram_tensor("b_inP", [128, 4], f32, kind="ExternalInput")
    a2_ds = {}
    a3_ds = {}
    for l in range(1, L):
        if l in DR3_LAYERS:
            a3_ds[l] = nc.dram_tensor(
                f"A3_{l}", [128, 2 * l, 2, 2, PAD], fp8, kind="ExternalInput"
            )
        else:
            a2_ds[l] = nc.dram_tensor(
                f"A2_{l}", [128, 2 * l, 2, PAD], fp8, kind="ExternalInput"
            )
    wout_d = nc.dram_tensor(
        "W_outT", [128, n_used, OUT_DIM], bf16, kind="ExternalInput"
    )
    bout_d = nc.dram_tensor("b_outP", [OUT_DIM, 1], f32, kind="ExternalOutput"
                            if False else "ExternalInput")
    out_d = nc.dram_tensor("out", [OUT_DIM, BL], f32, kind="ExternalOutput")

    with tile.TileContext(nc) as tc:
        with (
            tc.tile_pool(name="const", bufs=1) as cpool,
            tc.tile_pool(name="hbuf", bufs=1) as hpool,
            tc.tile_pool(name="ps", bufs=7, space="PSUM") as ppool,
            tc.tile_pool(name="pso", bufs=1, space="PSUM") as opool,
            tc.tile_pool(name="outs", bufs=1) as spool,
        ):
            # ---- DMA issue: alternate SP/Act queues so per-instruction
            # setup (seq 565-667ns, HWDGE 625ns) pipelines; small tensors
            # ride mid-stream so the A stream starts as early as possible.
            def dma(dst, src):
                nc.sync.dma_start(dst, src)

            win_tiles = [cpool.tile([128, 7, 128], bf16, name=f"win{m}")
                         for m in range(4)]
            xt_s = cpool.tile([128, 7, BL], bf16)
            dma(win_tiles[0][:], win_d[0])
            dma(xt_s[:, 0:2, :], xT_d[:, 0:2, :])
            dma(xt_s[:, 2:7, :], xT_d[:, 2:7, :])
            for m in range(1, 4):
                dma(win_tiles[m][:], win_d[m])
            bin_s = cpool.tile([128, 4], f32)
            wout_s = cpool.tile([128, n_used, OUT_DIM], bf16)
            bout_s = cpool.tile([OUT_DIM, 1], f32)

            # ---- A stream: whole-layer tiles for DR2, chunked for DR3 ----
            a2_s = {}
            a3_chunks = {}
            for l in range(1, L):
                if l not in DR3_LAYERS:
                    t = cpool.tile([128, 2 * l, 2, PAD], fp8, name=f"a2_{l}")
                    half = l  # split layer DMA in two for pipelining
                    dma(t[:, 0:half], a2_ds[l][:, 0:half])
                    dma(t[:, half:2 * l], a2_ds[l][:, half:2 * l])
                    a2_s[l] = t
                else:
                    bounds = list(range(0, 2 * l - 2, DR3_CHUNK_KP))
                    bounds += [2 * l - 2, 2 * l - 1, 2 * l]
                    chunks = []
                    for c0, c1 in zip(bounds[:-1], bounds[1:]):
                        t = cpool.tile(
                            [128, c1 - c0, 2, 2, PAD], fp8, name=f"a3_{l}_{c0}"
                        )
                        dma(t[:], a3_ds[l][:, c0:c1])
                        chunks.append((c0, c1, t))
                    a3_chunks[l] = chunks
                if l == 1:
                    dma(bin_s[:], bin_d[:])
                elif l == 4:
                    dma(wout_s[:], wout_d[:])
                    dma(bout_s[:], bout_d[:])

            # ---- activation storage: one tile set per layer for precise
            # dependency tracking (a matmul reading layer j's h only waits
            # on layer j's activation writes, not the latest layer's) ----
            h_bf_t = [hpool.tile([128, 4, BL], bf16, name=f"hbf{j}")
                      for j in range(L)]
            h_hi_t = [hpool.tile([128, 4, BL], fp8, name=f"hhi{j}")
                      for j in range(L)]
            h_lo_t = [hpool.tile([128, 4, BL], fp8, name=f"hlo{j}")
                      for j in range(L)]
            zeros_s = cpool.tile([128, BL], f32)
            nc.vector.memset(zeros_s[:], 0.0)

            # ---- PE warmup: ~3.5us of dummy DoubleRow matmuls on zeroed
            # fp8 tiles keeps the clock-ramp model at full speed when the
            # first real matmul issues (otherwise the whole input layer
            # runs at the mid p-state).
            wu_w = cpool.tile([128, 2, 128], fp8, name="wu_w")
            wu_x = cpool.tile([128, 2, BL], fp8, name="wu_x")
            nc.vector.memset(wu_w[:], 0.0)
            nc.vector.memset(wu_x[:], 0.0)
            wu_ps = ppool.tile([128, BL], f32, tag="ps", name="wu_ps")
            for i in range(WARMUP_MM):
                nc.tensor.matmul(
                    wu_ps[:], wu_w[:], wu_x[:],
                    start=(i == 0), stop=(i == WARMUP_MM - 1), perf_mode=DR,
                )

            def rhs_slice(arr_t, kp):
                j, p = kp // 2, kp % 2
                return arr_t[j][:, 2 * p:2 * p + 2, :]

            def stt_relu(eng, out, ps, scale, bias):
                if isinstance(bias, float):
                    eng.scalar_tensor_tensor(
                        out, ps[:], scale, zeros_s[:],
                        mybir.AluOpType.mult, mybir.AluOpType.max,
                    )
                else:
                    eng.scalar_tensor_tensor(
                        out, ps[:], bias, zeros_s[:],
                        mybir.AluOpType.add, mybir.AluOpType.max,
                    )

            def split_layer(j, psums, scale, biases):
                """4 psums -> h_hi (Act m0,2 / DVE m1,3), h_bf, h_lo.

                h_hi lands fast (parity-split across two engines) since the
                next layer's first matmuls need it in m order; h_bf next
                (frees psums); h_lo last (only needed at the end of the
                next layer).
                """
                for m in (0, 2):
                    nc.scalar.activation(
                        h_hi_t[j][:, m, :], psums[m][:], AF.Relu,
                        bias=biases[m], scale=scale,
                    )
                for m in (1, 3):
                    stt_relu(nc.vector, h_hi_t[j][:, m, :], psums[m],
                             scale, biases[m])
                for m in (1, 3):
                    nc.scalar.activation(
                        h_bf_t[j][:, m, :], psums[m][:], AF.Relu,
                        bias=biases[m], scale=scale,
                    )
                for m in (0, 2):
                    stt_relu(nc.vector, h_bf_t[j][:, m, :], psums[m],
                             scale, biases[m])
                for m in range(4):
                    nc.vector.tensor_sub(
                        h_lo_t[j][:, m, :], h_bf_t[j][:, m, :],
                        h_hi_t[j][:, m, :]
                    )

            def bf_only(j, m, ps, scale, bias):
                """Last layer: only h_bf is consumed (by the head)."""
                if m in (0, 2):
                    nc.scalar.activation(
                        h_bf_t[j][:, m, :], ps[:], AF.Relu,
                        bias=bias, scale=scale,
                    )
                else:
                    stt_relu(nc.vector, h_bf_t[j][:, m, :], ps, scale, bias)

            # head bookkeeping: emit used-tile matmuls as soon as the
            # owning layer's h_bf lands
            used_by_layer = {}
            for i, kt in enumerate(used_tiles):
                used_by_layer.setdefault(kt // 4, []).append((i, kt))
            pso = opool.tile([OUT_DIM, BL], f32)
            head_emitted = [0]

            def emit_head(j, ms=None):
                for i, kt in used_by_layer.get(j, []):
                    if ms is not None and (kt % 4) not in ms:
                        continue
                    nc.tensor.matmul(
                        pso[:],
                        wout_s[:, i, :],
                        h_bf_t[j][:, kt % 4, :],
                        start=(head_emitted[0] == 0),
                        stop=(head_emitted[0] == len(used_tiles) - 1),
                    )
                    head_emitted[0] += 1

            # ---- input layer: h[0:4] = relu(W_in.T.T @ xT + b_in) ----
            pins = [ppool.tile([128, BL], f32, tag="ps", name=f"pin{m}")
                    for m in range(4)]
            for m in range(4):
                for kt in range(7):
                    nc.tensor.matmul(
                        pins[m][:],
                        win_tiles[m][:, kt, :],
                        xt_s[:, kt, :],
                        start=(kt == 0),
                        stop=(kt == 6),
                    )
            split_layer(0, pins, 1.0, [bin_s[:, m:m + 1] for m in range(4)])
            emit_head(0)

            # ---- hidden layers (fp8 DoubleRow) ----
            for l in range(1, L):
                nkp = 2 * l
                pls = [
                    ppool.tile([128, BL], f32, tag="ps", name=f"pl{l}_{m}")
                    for m in range(4)
                ]
                n_passes = 3 if l in DR3_LAYERS else 2
                per_m = nkp * n_passes  # accumulation-group length per psum
                idx = 0  # index within each m's group (same for all m)
                # h_hi passes stream with the A chunks; the h_lo pass runs
                # last so the DVE has the whole layer to produce h_lo of the
                # just-finished previous layer.
                if l in DR3_LAYERS:
                    for c0, c1, t in a3_chunks[l]:
                        for kp in range(c0, c1):
                            ci = kp - c0
                            rhs_hi = rhs_slice(h_hi_t, kp)
                            for w_sel in (0, 1):
                                for m in range(4):
                                    nc.tensor.matmul(
                                        pls[m][:],
                                        t[:, ci, w_sel, :, m * 128:(m + 1) * 128],
                                        rhs_hi,
                                        start=(idx == 0),
                                        stop=(idx == per_m - 1),
                                        perf_mode=DR,
                                    )
                                idx += 1
                    for c0, c1, t in a3_chunks[l]:
                        for kp in range(c0, c1):
                            ci = kp - c0
                            rhs_lo = rhs_slice(h_lo_t, kp)
                            for m in range(4):
                                nc.tensor.matmul(
                                    pls[m][:],
                                    t[:, ci, 0, :, m * 128:(m + 1) * 128],
                                    rhs_lo,
                                    start=(idx == 0),
                                    stop=(idx == per_m - 1),
                                    perf_mode=DR,
                                )
                            idx += 1
                else:
                    t = a2_s[l]
                    for kp in range(nkp):
                        rhs_hi = rhs_slice(h_hi_t, kp)
                        for m in range(4):
                            nc.tensor.matmul(
                                pls[m][:],
                                t[:, kp, :, m * 128:(m + 1) * 128],
                                rhs_hi,
                                start=(idx == 0),
                                stop=(idx == per_m - 1),
                                perf_mode=DR,
                            )
                        idx += 1
                    for kp in range(nkp):
                        rhs_lo = rhs_slice(h_lo_t, kp)
                        for m in range(4):
                            nc.tensor.matmul(
                                pls[m][:],
                                t[:, kp, :, m * 128:(m + 1) * 128],
                                rhs_lo,
                                start=(idx == 0),
                                stop=(idx == per_m - 1),
                                perf_mode=DR,
                            )
                        idx += 1
                if l == L - 1:
                    for m in range(4):
                        bf_only(l, m, pls[m], inv_s, 0.0)
                else:
                    split_layer(l, pls, inv_s, [0.0] * 4)
                emit_head(l)

            # ---- output head epilogue ----
            out_s = spool.tile([OUT_DIM, BL], f32)
            nc.scalar.activation(out_s[:], pso[:], AF.Identity, bias=bout_s[:])
            nc.sync.dma_start(out_d[:], out_s[:])

    nc.compile()
    return nc


def _pack_ptiles(arr2d, n_tiles):
    """[n_tiles*128, F] row-major -> [128, n_tiles, F] partition-major."""
    f = arr2d.shape[1]
    return np.ascontiguousarray(
        arr2d.reshape(n_tiles, 128, f).transpose(1, 0, 2)
    )


def kernel(**inputs):
    x = np.asarray(inputs["x"], np.float32)
    W_in = np.asarray(inputs["W_in"], np.float32)
    b_in = np.asarray(inputs["b_in"], np.float32)
    w_edge = np.asarray(inputs["w_edge"], np.float32)
    W_out = np.asarray(inputs["W_out"], np.float32)
    b_out = np.asarray(inputs["b_out"], np.float32)
    edge_src = np.asarray(inputs["edge_src"]).astype(np.int64)
    edge_dst = np.asarray(inputs["edge_dst_local"]).astype(np.int64)
    offsets = np.asarray(inputs["edge_offsets"]).astype(np.int64)
    out_verts = np.asarray(inputs["out_verts"]).astype(np.int64)

    # ---- host-side packing ----
    shared = {}
    for l in range(1, L):
        s, e = int(offsets[l - 1]), int(offsets[l])
        At = np.zeros((l * PAD, PAD), np.float32)  # [src_padded, tgt]
        rows = (edge_src[s:e] // PER) * PAD + (edge_src[s:e] % PER)
        np.add.at(At, (rows, edge_dst[s:e]), w_edge[s:e])
        At *= A_SCALE
        A_hi = At.astype(float8_e4m3)
        A_lo = (At - A_hi.astype(np.float32)).astype(float8_e4m3)
        # [4l*128, 512] -> [2l kp, 2 kt, 128, 512] -> [128, 2l, 2, 512]
        def kp_form(a8):
            return np.ascontiguousarray(
                a8.reshape(2 * l, 2, 128, PAD).transpose(2, 0, 1, 3)
            )
        hi = kp_form(A_hi)
        if l in DR3_LAYERS:
            lo = kp_form(A_lo)
            # [128, 2l, 2(hilo), 2, 512]
            shared[f"A3_{l}"] = np.ascontiguousarray(
                np.stack([hi, lo], axis=2)
            )
        else:
            shared[f"A2_{l}"] = hi

    winT = np.zeros((K_IN, PAD), np.float32)
    winT[:IN_DIM, :PER] = W_in.T
    winT_re = np.ascontiguousarray(
        _pack_ptiles(winT, 7).reshape(128, 7, 4, 128).transpose(2, 0, 1, 3)
    ).astype(bfloat16)

    binP = np.zeros((PAD,), np.float32)
    binP[:PER] = b_in
    binP_re = np.ascontiguousarray(binP.reshape(4, 128).T)

    woutT = np.zeros((NT * 128, OUT_DIM), np.float32)
    pad_idx = (out_verts // PER) * PAD + (out_verts % PER)
    woutT[pad_idx, :] = W_out.T
    used_tiles = tuple(sorted(set(int(t) for t in pad_idx // 128)))
    woutT_re = np.ascontiguousarray(
        _pack_ptiles(woutT, NT)[:, list(used_tiles), :]
    ).astype(bfloat16)

    boutP = np.ascontiguousarray(b_out.reshape(OUT_DIM, 1))

    shared.update({
        "W_inT": winT_re,
        "b_inP": binP_re,
        "W_outT": woutT_re,
        "b_outP": boutP,
    })
    in_maps = []
    for c in range(NC):
        xT = np.zeros((K_IN, BL), np.float32)
        xT[:IN_DIM, :] = x[c * BL:(c + 1) * BL, :].T
        in_maps.append({"xT": _pack_ptiles(xT, 7).astype(bfloat16), **shared})

    from concourse.bass_utils import run_bass_kernel_spmd

    global _LAST_IN_MAPS, _PROG, _PROG_KEY
    _LAST_IN_MAPS = in_maps
    if _PROG is None or _PROG_KEY != used_tiles:
        _PROG = _build_program(used_tiles)
        _PROG_KEY = used_tiles
    res = run_bass_kernel_spmd(_PROG, in_maps, list(range(NC)))
    out = np.concatenate(
        [np.asarray(res.results[c]["out"], np.float32).T for c in range(NC)], axis=0
    )
    return np.ascontiguousarray(out)


# revision 14
# speedup vs baseline: 1.3697x; 1.0109x over previous
"""GNN message-passing kernel for Trainium2 (Bass/Tile), 8-core SPMD.

Model (from the reference):
  h0 = relu(x @ W_in.T + b_in).T            # [500, B] -> vertices 0..500
  for l in 1..7:   agg = segment_sum(w_edge * h[edge_src]) ; h_l = relu(agg)
  out = h[out_verts].T @ W_out.T + b_out    # [B, 10]

Device strategy:
  - Data-parallel over batch: 8 cores x 256 columns each.
  - The sparse aggregation is a dense matmul agg = A_l @ h_lower with
    A_l built on the host. A is streamed in fp8(e4m3) and the matmuls
    run in DoubleRow perf mode (256-deep contraction per instruction at
    0.5 cyc/row), which is ~4x the bf16 MAC rate.
  - Precision: A is scaled by 16 and split into hi (+ lo residual on
    late layers); activations h are kept as a bf16 master copy plus an
    fp8 hi/lo pair. Per layer l the aggregation computes
        DR2:  A_hi @ (h_hi + h_lo)              (layers 1..5)
        DR3:  A_hi @ (h_hi + h_lo) + A_lo @ h_hi (layers 6..7)
    Late layers get the A residual because their error feeds the output
    head directly; early-layer errors wash out through the 32-edge
    averaging of subsequent layers.
  - Input layer and output head stay bf16.
  - Vertex space padded to 512/layer: every layer is 4 partition tiles
    of 128, and DoubleRow pairs two 128-tiles per instruction.
  - The out_verts gather is folded into a scattered W_out on the host.
"""

import sys

try:
    import concourse  # noqa: F401  (provided by the axon site-path)
except ImportError:
    sys.path.insert(0, "/opt/trn_rl_repo")

import numpy as np
from ml_dtypes import bfloat16, float8_e4m3

# ---- problem geometry (fixed by the problem spec) ----
B = 2048            # total batch
NC = 8              # cores
BL = B // NC        # 256 batch columns per core
IN_DIM = 784
K_IN = 896          # 784 padded to 7*128
PER = 500           # vertices per layer
PAD = 512           # padded vertices per layer (4*128)
L = 8               # layers (layer 0 = input layer)
NT = 4 * L          # 32 h tiles of 128 vertices
OUT_DIM = 10

A_SCALE = 16.0      # fp8 subnormal rescue; undone by the act scale=1/16
# layers carrying the A_lo residual correction (DR3); others are DR2
DR3_LAYERS = (6, 7)
# DMA chunk size for DR3 layers, in k-pairs (bounds the tail exposure)
DR3_CHUNK_KP = 2
WARMUP_MM = 22

_PROG = None
_PROG_KEY = None
_LAST_IN_MAPS = None  # kept for external profiling harnesses


def _build_program(used_tiles):
    from concourse import bacc, tile
    import concourse.mybir as mybir

    f32 = mybir.dt.float32
    bf16 = mybir.dt.bfloat16
    fp8 = mybir.dt.float8e4
    AF = mybir.ActivationFunctionType
    DR = mybir.MatmulPerfMode.DoubleRow

    n_used = len(used_tiles)
    inv_s = 1.0 / A_SCALE  # noqa - WARMUP_MM from module scope
    nc = bacc.Bacc(None, target_bir_lowering=False)

    xT_d = nc.dram_tensor("xT", [128, 7, BL], bf16, kind="ExternalInput")
    win_d = nc.dram_tensor("W_inT", [4, 128, 7, 128], bf16, kind="ExternalInput")
    bin_d = nc.dram_tensor("b_inP", [128, 4], f32, kind="ExternalInput")
    a2_ds = {}
    a3_ds = {}
    for l in range(1, L):
        if l in DR3_LAYERS:
            a3_ds[l] = nc.dram_tensor(
                f"A3_{l}", [128, 2 * l, 2, 2, PAD], fp8, kind="ExternalInput"
            )
        else:
            a2_ds[l] = nc.dram_tensor(
                f"A2_{l}", [128, 2 * l, 2, PAD], fp8, kind="ExternalInput"
            )
    wout_d = nc.dram_tensor(
        "W_outT", [128, n_used, OUT_DIM], bf16, kind="ExternalInput"
    )
    bout_d = nc.dram_tensor("b_outP", [OUT_DIM, 1], f32, kind="ExternalOutput"
                            if False else "ExternalInput")
    out_d = nc.dram_tensor("out", [OUT_DIM, BL], f32, kind="ExternalOutput")

    with tile.TileContext(nc) as tc:
        with (
            tc.tile_pool(name="const", bufs=1) as cpool,
            tc.tile_pool(name="hbuf", bufs=1) as hpool,
            tc.tile_pool(name="ps", bufs=7, space="PSUM") as ppool,
            tc.tile_pool(name="pso", bufs=1, space="PSUM") as opool,
            tc.tile_pool(name="outs", bufs=1) as spool,
        ):
            # ---- DMA issue: alternate SP/Act queues so per-instruction
            # setup (seq 565-667ns, HWDGE 625ns) pipelines; small tensors
            # ride mid-stream so the A stream starts as early as possible.
            def dma(dst, src):
                nc.sync.dma_start(dst, src)

            win_tiles = [cpool.tile([128, 7, 128], bf16, name=f"win{m}")
                         for m in range(4)]
            xt_s = cpool.tile([128, 7, BL], bf16)
            dma(win_tiles[0][:], win_d[0])
            dma(xt_s[:, 0:2, :], xT_d[:, 0:2, :])
            dma(xt_s[:, 2:7, :], xT_d[:, 2:7, :])
            for m in range(1, 4):
                dma(win_tiles[m][:], win_d[m])
            bin_s = cpool.tile([128, 4], f32)
            wout_s = cpool.tile([128, n_used, OUT_DIM], bf16)
            bout_s = cpool.tile([OUT_DIM, 1], f32)

            # ---- A stream: whole-layer tiles for DR2, chunked for DR3 ----
            a2_s = {}
            a3_chunks = {}
            for l in range(1, L):
                if l not in DR3_LAYERS:
                    t = cpool.tile([128, 2 * l, 2, PAD], fp8, name=f"a2_{l}")
                    half = l  # split layer DMA in two for pipelining
                    dma(t[:, 0:half], a2_ds[l][:, 0:half])
                    dma(t[:, half:2 * l], a2_ds[l][:, half:2 * l])
                    a2_s[l] = t
                else:
                    bounds = list(range(0, 2 * l - 2, DR3_CHUNK_KP))
                    bounds += [2 * l - 2, 2 * l - 1, 2 * l]
                    chunks = []
                    for c0, c1 in zip(bounds[:-1], bounds[1:]):
                        t = cpool.tile(
                            [128, c1 - c0, 2, 2, PAD], fp8, name=f"a3_{l}_{c0}"
                        )
                        dma(t[:], a3_ds[l][:, c0:c1])
                        chunks.append((c0, c1, t))
                    a3_chunks[l] = chunks
                if l == 1:
                    dma(bin_s[:], bin_d[:])
                elif l == 4:
                    dma(wout_s[:], wout_d[:])
                    dma(bout_s[:], bout_d[:])

            # ---- activation storage: one tile set per layer for precise
            # dependency tracking (a matmul reading layer j's h only waits
            # on layer j's activation writes, not the latest layer's) ----
            h_bf_t = [hpool.tile([128, 4, BL], bf16, name=f"hbf{j}")
                      for j in range(L)]
            h_hi_t = [hpool.tile([128, 4, BL], fp8, name=f"hhi{j}")
                      for j in range(L)]
            h_lo_t = [hpool.tile([128, 4, BL], fp8, name=f"hlo{j}")
                      for j in range(L)]
            zeros_s = cpool.tile([128, BL], f32)
            nc.vector.memset(zeros_s[:], 0.0)

            # ---- PE warmup: ~3.5us of dummy DoubleRow matmuls on zeroed
            # fp8 tiles keeps the clock-ramp model at full speed when the
            # first real matmul issues (otherwise the whole input layer
            # runs at the mid p-state).
            wu_w = cpool.tile([128, 2, 128], fp8, name="wu_w")
            wu_x = cpool.tile([128, 2, BL], fp8, name="wu_x")
            nc.vector.memset(wu_w[:], 0.0)
            nc.vector.memset(wu_x[:], 0.0)
            wu_ps = ppool.tile([128, BL], f32, tag="ps", name="wu_ps")
            for i in range(WARMUP_MM):
                nc.tensor.matmul(
                    wu_ps[:], wu_w[:], wu_x[:],
                    start=(i == 0), stop=(i == WARMUP_MM - 1), perf_mode=DR,
                )

            def rhs_slice(arr_t, kp):
                j, p = kp // 2, kp % 2
                return arr_t[j][:, 2 * p:2 * p + 2, :]

            def stt_relu(eng, out, ps, scale, bias):
                if isinstance(bias, float):
                    eng.scalar_tensor_tensor(
                        out, ps[:], scale, zeros_s[:],
                        mybir.AluOpType.mult, mybir.AluOpType.max,
                    )
                else:
                    eng.scalar_tensor_tensor(
                        out, ps[:], bias, zeros_s[:],
                        mybir.AluOpType.add, mybir.AluOpType.max,
                    )

            def split_layer(j, psums, scale, biases):
                """4 psums -> h_hi (Act m0,2 / DVE m1,3), h_bf, h_lo.

                h_hi lands fast (parity-split across two engines) since the
                next layer's first matmuls need it in m order; h_bf next
                (frees psums); h_lo last (only needed at the end of the
                next layer).
                """
                for m in (0, 2):
                    nc.scalar.activation(
                        h_hi_t[j][:, m, :], psums[m][:], AF.Relu,
                        bias=biases[m], scale=scale,
                    )
                for m in (1, 3):
                    stt_relu(nc.vector, h_hi_t[j][:, m, :], psums[m],
                             scale, biases[m])
                for m in (0, 2):
                    nc.scalar.activation(
                        h_bf_t[j][:, m, :], psums[m][:], AF.Relu,
                        bias=biases[m], scale=scale,
                    )
                for m in (1, 3):
                    stt_relu(nc.vector, h_bf_t[j][:, m, :], psums[m],
                             scale, biases[m])
                for m in range(4):
                    nc.vector.tensor_sub(
                        h_lo_t[j][:, m, :], h_bf_t[j][:, m, :],
                        h_hi_t[j][:, m, :]
                    )

            def bf_only(j, m, ps, scale, bias):
                """Last layer: only h_bf is consumed (by the head)."""
                if m in (0, 2):
                    nc.scalar.activation(
                        h_bf_t[j][:, m, :], ps[:], AF.Relu,
                        bias=bias, scale=scale,
                    )
                else:
                    stt_relu(nc.vector, h_bf_t[j][:, m, :], ps, scale, bias)

            # head bookkeeping: emit used-tile matmuls as soon as the
            # owning layer's h_bf lands
            used_by_layer = {}
            for i, kt in enumerate(used_tiles):
                used_by_layer.setdefault(kt // 4, []).append((i, kt))
            pso = opool.tile([OUT_DIM, BL], f32)
            head_emitted = [0]

            def emit_head(j, ms=None):
                for i, kt in used_by_layer.get(j, []):
                    if ms is not None and (kt % 4) not in ms:
                        continue
                    nc.tensor.matmul(
                        pso[:],
                        wout_s[:, i, :],
                        h_bf_t[j][:, kt % 4, :],
                        start=(head_emitted[0] == 0),
                        stop=(head_emitted[0] == len(used_tiles) - 1),
                    )
                    head_emitted[0] += 1

            # ---- input layer: h[0:4] = relu(W_in.T.T @ xT + b_in) ----
            pins = [ppool.tile([128, BL], f32, tag="ps", name=f"pin{m}")
                    for m in range(4)]
            for m in range(4):
                for kt in range(7):
                    nc.tensor.matmul(
                        pins[m][:],
                        win_tiles[m][:, kt, :],
                        xt_s[:, kt, :],
                        start=(kt == 0),
                        stop=(kt == 6),
                    )
            split_layer(0, pins, 1.0, [bin_s[:, m:m + 1] for m in range(4)])
            emit_head(0)

            # ---- hidden layers (fp8 DoubleRow) ----
            for l in range(1, L):
                nkp = 2 * l
                pls = [
                    ppool.tile([128, BL], f32, tag="ps", name=f"pl{l}_{m}")
                    for m in range(4)
                ]
                n_passes = 3 if l in DR3_LAYERS else 2
                per_m = nkp * n_passes  # accumulation-group length per psum
                idx = 0  # index within each m's group (same for all m)
                # h_hi passes stream with the A chunks; the h_lo pass runs
                # last so the DVE has the whole layer to produce h_lo of the
                # just-finished previous layer.
                if l in DR3_LAYERS:
                    for c0, c1, t in a3_chunks[l]:
                        for kp in range(c0, c1):
                            ci = kp - c0
                            rhs_hi = rhs_slice(h_hi_t, kp)
                            for w_sel in (0, 1):
                                for m in range(4):
                                    nc.tensor.matmul(
                                        pls[m][:],
                                        t[:, ci, w_sel, :, m * 128:(m + 1) * 128],
                                        rhs_hi,
                                        start=(idx == 0),
                                        stop=(idx == per_m - 1),
                                        perf_mode=DR,
                                    )
                                idx += 1
                    for c0, c1, t in a3_chunks[l]:
                        for kp in range(c0, c1):
                            ci = kp - c0
                            rhs_lo = rhs_slice(h_lo_t, kp)
                            for m in range(4):
                                nc.tensor.matmul(
                                    pls[m][:],
                                    t[:, ci, 0, :, m * 128:(m + 1) * 128],
                                    rhs_lo,
                                    start=(idx == 0),
                                    stop=(idx == per_m - 1),
                                    perf_mode=DR,
                                )
                            idx += 1
                else:
                    t = a2_s[l]
                    for kp in range(nkp):
                        rhs_hi = rhs_slice(h_hi_t, kp)
                        for m in range(4):
                            nc.tensor.matmul(
                                pls[m][:],
                                t[:, kp, :, m * 128:(m + 1) * 128],
                                rhs_hi,
                                start=(idx == 0),
                                stop=(idx == per_m - 1),
                                perf_mode=DR,
                            )
                        idx += 1
                    for kp in range(nkp):
                        rhs_lo = rhs_slice(h_lo_t, kp)
                        for m in range(4):
                            nc.tensor.matmul(
                                pls[m][:],
                                t[:, kp, :, m * 128:(m + 1) * 128],
                                rhs_lo,
                                start=(idx == 0),
                                stop=(idx == per_m - 1),
                                perf_mode=DR,
                            )
                        idx += 1
                if l == L - 1:
                    for m in range(4):
                        bf_only(l, m, pls[m], inv_s, 0.0)
                else:
                    split_layer(l, pls, inv_s, [0.0] * 4)
                emit_head(l)

            # ---- output head epilogue ----
            out_s = spool.tile([OUT_DIM, BL], f32)
            nc.scalar.activation(out_s[:], pso[:], AF.Identity, bias=bout_s[:])
            nc.sync.dma_start(out_d[:], out_s[:])

    nc.compile()
    return nc


def _pack_ptiles(arr2d, n_tiles):
    """[n_tiles*128, F] row-major -> [128, n_tiles, F] partition-major."""
    f = arr2d.shape[1]
    return np.ascontiguousarray(
        arr2d.reshape(n_tiles, 128, f).transpose(1, 0, 2)
    )


def kernel(**inputs):
    x = np.asarray(inputs["x"], np.float32)
    W_in = np.asarray(inputs["W_in"], np.float32)
    b_in = np.asarray(inputs["b_in"], np.float32)
    w_edge = np.asarray(inputs["w_edge"], np.float32)
    W_out = np.asarray(inputs["W_out"], np.float32)
    b_out = np.asarray(inputs["b_out"], np.float32)
    edge_src = np.asarray(inputs["edge_src"]).astype(np.int64)
    edge_dst = np.asarray(inputs["edge_dst_local"]).astype(np.int64)
    offsets = np.asarray(inputs["edge_offsets"]).astype(np.int64)
    out_verts = np.asarray(inputs["out_verts"]).astype(np.int64)

    # ---- host-side packing ----
    shared = {}
    for l in range(1, L):
        s, e = int(offsets[l - 1]), int(offsets[l])
        At = np.zeros((l * PAD, PAD), np.float32)  # [src_padded, tgt]
        rows = (edge_src[s:e] // PER) * PAD + (edge_src[s:e] % PER)
        np.add.at(At, (rows, edge_dst[s:e]), w_edge[s:e])
        At *= A_SCALE
        A_hi = At.astype(float8_e4m3)
        A_lo = (At - A_hi.astype(np.float32)).astype(float8_e4m3)
        # [4l*128, 512] -> [2l kp, 2 kt, 128, 512] -> [128, 2l, 2, 512]
        def kp_form(a8):
            return np.ascontiguousarray(
                a8.reshape(2 * l, 2, 128, PAD).transpose(2, 0, 1, 3)
            )
        hi = kp_form(A_hi)
        if l in DR3_LAYERS:
            lo = kp_form(A_lo)
            # [128, 2l, 2(hilo), 2, 512]
            shared[f"A3_{l}"] = np.ascontiguousarray(
                np.stack([hi, lo], axis=2)
            )
        else:
            shared[f"A2_{l}"] = hi

    winT = np.zeros((K_IN, PAD), np.float32)
    winT[:IN_DIM, :PER] = W_in.T
    winT_re = np.ascontiguousarray(
        _pack_ptiles(winT, 7).reshape(128, 7, 4, 128).transpose(2, 0, 1, 3)
    ).astype(bfloat16)

    binP = np.zeros((PAD,), np.float32)
    binP[:PER] = b_in
    binP_re = np.ascontiguousarray(binP.reshape(4, 128).T)

    woutT = np.zeros((NT * 128, OUT_DIM), np.float32)
    pad_idx = (out_verts // PER) * PAD + (out_verts % PER)
    woutT[pad_idx, :] = W_out.T
    used_tiles = tuple(sorted(set(int(t) for t in pad_idx // 128)))
    woutT_re = np.ascontiguousarray(
        _pack_ptiles(woutT, NT)[:, list(used_tiles), :]
    ).astype(bfloat16)

    boutP = np.ascontiguousarray(b_out.reshape(OUT_DIM, 1))

    shared.update({
        "W_inT": winT_re,
        "b_inP": binP_re,
        "W_outT": woutT_re,
        "b_outP": boutP,
    })
    in_maps = []
    for c in range(NC):
        xT = np.zeros((K_IN, BL), np.float32)
        xT[:IN_DIM, :] = x[c * BL:(c + 1) * BL, :].T
        in_maps.append({"xT": _pack_ptiles(xT, 7).astype(bfloat16), **shared})

    from concourse.bass_utils import run_bass_kernel_spmd

    global _LAST_IN_MAPS, _PROG, _PROG_KEY
    _LAST_IN_MAPS = in_maps
    if _PROG is None or _PROG_KEY != used_tiles:
        _PROG = _build_program(used_tiles)
        _PROG_KEY = used_tiles
    res = run_bass_kernel_spmd(_PROG, in_maps, list(range(NC)))
    out = np.concatenate(
        [np.asarray(res.results[c]["out"], np.float32).T for c in range(NC)], axis=0
    )
    return np.ascontiguousarray(out)


# revision 15
# speedup vs baseline: 1.4487x; 1.0577x over previous
"""GNN message-passing kernel for Trainium2 (Bass/Tile), 8-core SPMD.

Model (from the reference):
  h0 = relu(x @ W_in.T + b_in).T            # [500, B] -> vertices 0..500
  for l in 1..7:   agg = segment_sum(w_edge * h[edge_src]) ; h_l = relu(agg)
  out = h[out_verts].T @ W_out.T + b_out    # [B, 10]

Device strategy:
  - Data-parallel over batch: 8 cores x 256 columns each.
  - The sparse aggregation is a dense matmul agg = A_l @ h_lower with
    A_l built on the host. A is streamed in fp8(e4m3) and the matmuls
    run in DoubleRow perf mode (256-deep contraction per instruction at
    0.5 cyc/row), which is ~4x the bf16 MAC rate.
  - Precision: A is scaled by 16 and split into hi (+ lo residual on
    late layers); activations h are kept as a bf16 master copy plus an
    fp8 hi/lo pair. Per layer l the aggregation computes
        DR2:  A_hi @ (h_hi + h_lo)              (layers 1..5)
        DR3:  A_hi @ (h_hi + h_lo) + A_lo @ h_hi (layers 6..7)
    Late layers get the A residual because their error feeds the output
    head directly; early-layer errors wash out through the 32-edge
    averaging of subsequent layers.
  - Input layer and output head stay bf16.
  - Vertex space padded to 512/layer: every layer is 4 partition tiles
    of 128, and DoubleRow pairs two 128-tiles per instruction.
  - The out_verts gather is folded into a scattered W_out on the host.
"""

import sys

try:
    import concourse  # noqa: F401  (provided by the axon site-path)
except ImportError:
    sys.path.insert(0, "/opt/trn_rl_repo")

import numpy as np
from ml_dtypes import bfloat16, float8_e4m3

# ---- problem geometry (fixed by the problem spec) ----
B = 2048            # total batch
NC = 8              # cores
BL = B // NC        # 256 batch columns per core
IN_DIM = 784
K_IN = 896          # 784 padded to 7*128
PER = 500           # vertices per layer
PAD = 512           # padded vertices per layer (4*128)
L = 8               # layers (layer 0 = input layer)
NT = 4 * L          # 32 h tiles of 128 vertices
OUT_DIM = 10

A_SCALE = 16.0      # fp8 subnormal rescue; undone by the act scale=1/16
# layers carrying the A_lo residual correction (DR3); others are DR2
DR3_LAYERS = (7,)
# DMA chunk size for DR3 layers, in k-pairs (bounds the tail exposure)
DR3_CHUNK_KP = 2
WARMUP_MM = 22

_PROG = None
_PROG_KEY = None
_LAST_IN_MAPS = None  # kept for external profiling harnesses


def _build_program(used_tiles):
    from concourse import bacc, tile
    import concourse.mybir as mybir

    f32 = mybir.dt.float32
    bf16 = mybir.dt.bfloat16
    fp8 = mybir.dt.float8e4
    AF = mybir.ActivationFunctionType
    DR = mybir.MatmulPerfMode.DoubleRow

    n_used = len(used_tiles)
    inv_s = 1.0 / A_SCALE  # noqa - WARMUP_MM from module scope
    nc = bacc.Bacc(None, target_bir_lowering=False)

    xT_d = nc.dram_tensor("xT", [128, 7, BL], bf16, kind="ExternalInput")
    win_d = nc.dram_tensor("W_inT", [4, 128, 7, 128], bf16, kind="ExternalInput")
    bin_d = nc.dram_tensor("b_inP", [128, 4], f32, kind="ExternalInput")
    a2_ds = {}
    a3_ds = {}
    for l in range(1, L):
        if l in DR3_LAYERS:
            a3_ds[l] = nc.dram_tensor(
                f"A3_{l}", [128, 2 * l, 2, 2, PAD], fp8, kind="ExternalInput"
            )
        else:
            a2_ds[l] = nc.dram_tensor(
                f"A2_{l}", [128, 2 * l, 2, PAD], fp8, kind="ExternalInput"
            )
    wout_d = nc.dram_tensor(
        "W_outT", [128, n_used, OUT_DIM], bf16, kind="ExternalInput"
    )
    bout_d = nc.dram_tensor("b_outP", [OUT_DIM, 1], f32, kind="ExternalOutput"
                            if False else "ExternalInput")
    out_d = nc.dram_tensor("out", [OUT_DIM, BL], f32, kind="ExternalOutput")

    with tile.TileContext(nc) as tc:
        with (
            tc.tile_pool(name="const", bufs=1) as cpool,
            tc.tile_pool(name="hbuf", bufs=1) as hpool,
            tc.tile_pool(name="ps", bufs=7, space="PSUM") as ppool,
            tc.tile_pool(name="pso", bufs=1, space="PSUM") as opool,
            tc.tile_pool(name="outs", bufs=1) as spool,
        ):
            # ---- DMA issue: alternate SP/Act queues so per-instruction
            # setup (seq 565-667ns, HWDGE 625ns) pipelines; small tensors
            # ride mid-stream so the A stream starts as early as possible.
            def dma(dst, src):
                nc.sync.dma_start(dst, src)

            win_tiles = [cpool.tile([128, 7, 128], bf16, name=f"win{m}")
                         for m in range(4)]
            xt_s = cpool.tile([128, 7, BL], bf16)
            dma(win_tiles[0][:], win_d[0])
            dma(xt_s[:, 0:2, :], xT_d[:, 0:2, :])
            dma(xt_s[:, 2:7, :], xT_d[:, 2:7, :])
            for m in range(1, 4):
                dma(win_tiles[m][:], win_d[m])
            bin_s = cpool.tile([128, 4], f32)
            wout_s = cpool.tile([128, n_used, OUT_DIM], bf16)
            bout_s = cpool.tile([OUT_DIM, 1], f32)

            # ---- A stream: whole-layer tiles for DR2, chunked for DR3 ----
            a2_s = {}
            a3_chunks = {}
            for l in range(1, L):
                if l not in DR3_LAYERS:
                    t = cpool.tile([128, 2 * l, 2, PAD], fp8, name=f"a2_{l}")
                    half = l  # split layer DMA in two for pipelining
                    dma(t[:, 0:half], a2_ds[l][:, 0:half])
                    dma(t[:, half:2 * l], a2_ds[l][:, half:2 * l])
                    a2_s[l] = t
                else:
                    bounds = list(range(0, 2 * l - 2, DR3_CHUNK_KP))
                    bounds += [2 * l - 2, 2 * l - 1, 2 * l]
                    chunks = []
                    for c0, c1 in zip(bounds[:-1], bounds[1:]):
                        t = cpool.tile(
                            [128, c1 - c0, 2, 2, PAD], fp8, name=f"a3_{l}_{c0}"
                        )
                        dma(t[:], a3_ds[l][:, c0:c1])
                        chunks.append((c0, c1, t))
                    a3_chunks[l] = chunks
                if l == 1:
                    dma(bin_s[:], bin_d[:])
                elif l == 4:
                    dma(wout_s[:], wout_d[:])
                    dma(bout_s[:], bout_d[:])

            # ---- activation storage: one tile set per layer for precise
            # dependency tracking (a matmul reading layer j's h only waits
            # on layer j's activation writes, not the latest layer's) ----
            h_bf_t = [hpool.tile([128, 4, BL], bf16, name=f"hbf{j}")
                      for j in range(L)]
            h_hi_t = [hpool.tile([128, 4, BL], fp8, name=f"hhi{j}")
                      for j in range(L)]
            h_lo_t = [hpool.tile([128, 4, BL], fp8, name=f"hlo{j}")
                      for j in range(L)]
            zeros_s = cpool.tile([128, BL], f32)
            nc.vector.memset(zeros_s[:], 0.0)

            # ---- PE warmup: ~3.5us of dummy DoubleRow matmuls on zeroed
            # fp8 tiles keeps the clock-ramp model at full speed when the
            # first real matmul issues (otherwise the whole input layer
            # runs at the mid p-state).
            wu_w = cpool.tile([128, 2, 128], fp8, name="wu_w")
            wu_x = cpool.tile([128, 2, BL], fp8, name="wu_x")
            nc.vector.memset(wu_w[:], 0.0)
            nc.vector.memset(wu_x[:], 0.0)
            wu_ps = ppool.tile([128, BL], f32, tag="ps", name="wu_ps")
            for i in range(WARMUP_MM):
                nc.tensor.matmul(
                    wu_ps[:], wu_w[:], wu_x[:],
                    start=(i == 0), stop=(i == WARMUP_MM - 1), perf_mode=DR,
                )

            def rhs_slice(arr_t, kp):
                j, p = kp // 2, kp % 2
                return arr_t[j][:, 2 * p:2 * p + 2, :]

            def stt_relu(eng, out, ps, scale, bias):
                if isinstance(bias, float):
                    eng.scalar_tensor_tensor(
                        out, ps[:], scale, zeros_s[:],
                        mybir.AluOpType.mult, mybir.AluOpType.max,
                    )
                else:
                    eng.scalar_tensor_tensor(
                        out, ps[:], bias, zeros_s[:],
                        mybir.AluOpType.add, mybir.AluOpType.max,
                    )

            def split_layer(j, psums, scale, biases):
                """4 psums -> h_hi (Act m0,2 / DVE m1,3), h_bf, h_lo.

                h_hi lands fast (parity-split across two engines) since the
                next layer's first matmuls need it in m order; h_bf next
                (frees psums); h_lo last (only needed at the end of the
                next layer).
                """
                for m in (0, 2):
                    nc.scalar.activation(
                        h_hi_t[j][:, m, :], psums[m][:], AF.Relu,
                        bias=biases[m], scale=scale,
                    )
                for m in (1, 3):
                    stt_relu(nc.vector, h_hi_t[j][:, m, :], psums[m],
                             scale, biases[m])
                for m in (0, 2):
                    nc.scalar.activation(
                        h_bf_t[j][:, m, :], psums[m][:], AF.Relu,
                        bias=biases[m], scale=scale,
                    )
                for m in (1, 3):
                    stt_relu(nc.vector, h_bf_t[j][:, m, :], psums[m],
                             scale, biases[m])
                for m in range(4):
                    nc.vector.tensor_sub(
                        h_lo_t[j][:, m, :], h_bf_t[j][:, m, :],
                        h_hi_t[j][:, m, :]
                    )

            def bf_only(j, m, ps, scale, bias):
                """Last layer: only h_bf is consumed (by the head)."""
                if m in (0, 2):
                    nc.scalar.activation(
                        h_bf_t[j][:, m, :], ps[:], AF.Relu,
                        bias=bias, scale=scale,
                    )
                else:
                    stt_relu(nc.vector, h_bf_t[j][:, m, :], ps, scale, bias)

            # head bookkeeping: emit used-tile matmuls as soon as the
            # owning layer's h_bf lands
            used_by_layer = {}
            for i, kt in enumerate(used_tiles):
                used_by_layer.setdefault(kt // 4, []).append((i, kt))
            pso = opool.tile([OUT_DIM, BL], f32)
            head_emitted = [0]

            def emit_head(j, ms=None):
                for i, kt in used_by_layer.get(j, []):
                    if ms is not None and (kt % 4) not in ms:
                        continue
                    nc.tensor.matmul(
                        pso[:],
                        wout_s[:, i, :],
                        h_bf_t[j][:, kt % 4, :],
                        start=(head_emitted[0] == 0),
                        stop=(head_emitted[0] == len(used_tiles) - 1),
                    )
                    head_emitted[0] += 1

            # ---- input layer: h[0:4] = relu(W_in.T.T @ xT + b_in) ----
            pins = [ppool.tile([128, BL], f32, tag="ps", name=f"pin{m}")
                    for m in range(4)]
            for m in range(4):
                for kt in range(7):
                    nc.tensor.matmul(
                        pins[m][:],
                        win_tiles[m][:, kt, :],
                        xt_s[:, kt, :],
                        start=(kt == 0),
                        stop=(kt == 6),
                    )
            split_layer(0, pins, 1.0, [bin_s[:, m:m + 1] for m in range(4)])
            emit_head(0)

            # ---- hidden layers (fp8 DoubleRow) ----
            for l in range(1, L):
                nkp = 2 * l
                pls = [
                    ppool.tile([128, BL], f32, tag="ps", name=f"pl{l}_{m}")
                    for m in range(4)
                ]
                n_passes = 3 if l in DR3_LAYERS else 2
                per_m = nkp * n_passes  # accumulation-group length per psum
                idx = 0  # index within each m's group (same for all m)
                # h_hi passes stream with the A chunks; the h_lo pass runs
                # last so the DVE has the whole layer to produce h_lo of the
                # just-finished previous layer.
                if l in DR3_LAYERS:
                    for c0, c1, t in a3_chunks[l]:
                        for kp in range(c0, c1):
                            ci = kp - c0
                            rhs_hi = rhs_slice(h_hi_t, kp)
                            for w_sel in (0, 1):
                                for m in range(4):
                                    nc.tensor.matmul(
                                        pls[m][:],
                                        t[:, ci, w_sel, :, m * 128:(m + 1) * 128],
                                        rhs_hi,
                                        start=(idx == 0),
                                        stop=(idx == per_m - 1),
                                        perf_mode=DR,
                                    )
                                idx += 1
                    for c0, c1, t in a3_chunks[l]:
                        for kp in range(c0, c1):
                            ci = kp - c0
                            rhs_lo = rhs_slice(h_lo_t, kp)
                            for m in range(4):
                                nc.tensor.matmul(
                                    pls[m][:],
                                    t[:, ci, 0, :, m * 128:(m + 1) * 128],
                                    rhs_lo,
                                    start=(idx == 0),
                                    stop=(idx == per_m - 1),
                                    perf_mode=DR,
                                )
                            idx += 1
                else:
                    t = a2_s[l]
                    for kp in range(nkp):
                        rhs_hi = rhs_slice(h_hi_t, kp)
                        for m in range(4):
                            nc.tensor.matmul(
                                pls[m][:],
                                t[:, kp, :, m * 128:(m + 1) * 128],
                                rhs_hi,
                                start=(idx == 0),
                                stop=(idx == per_m - 1),
                                perf_mode=DR,
                            )
                        idx += 1
                    for kp in range(nkp):
                        rhs_lo = rhs_slice(h_lo_t, kp)
                        for m in range(4):
                            nc.tensor.matmul(
                                pls[m][:],
                                t[:, kp, :, m * 128:(m + 1) * 128],
                                rhs_lo,
                                start=(idx == 0),
                                stop=(idx == per_m - 1),
                                perf_mode=DR,
                            )
                        idx += 1
                if l == L - 1:
                    for m in range(4):
                        bf_only(l, m, pls[m], inv_s, 0.0)
                else:
                    split_layer(l, pls, inv_s, [0.0] * 4)
                emit_head(l)

            # ---- output head epilogue ----
            out_s = spool.tile([OUT_DIM, BL], f32)
            nc.scalar.activation(out_s[:], pso[:], AF.Identity, bias=bout_s[:])
            nc.sync.dma_start(out_d[:], out_s[:])

    nc.compile()
    return nc


def _pack_ptiles(arr2d, n_tiles):
    """[n_tiles*128, F] row-major -> [128, n_tiles, F] partition-major."""
    f = arr2d.shape[1]
    return np.ascontiguousarray(
        arr2d.reshape(n_tiles, 128, f).transpose(1, 0, 2)
    )


def kernel(**inputs):
    x = np.asarray(inputs["x"], np.float32)
    W_in = np.asarray(inputs["W_in"], np.float32)
    b_in = np.asarray(inputs["b_in"], np.float32)
    w_edge = np.asarray(inputs["w_edge"], np.float32)
    W_out = np.asarray(inputs["W_out"], np.float32)
    b_out = np.asarray(inputs["b_out"], np.float32)
    edge_src = np.asarray(inputs["edge_src"]).astype(np.int64)
    edge_dst = np.asarray(inputs["edge_dst_local"]).astype(np.int64)
    offsets = np.asarray(inputs["edge_offsets"]).astype(np.int64)
    out_verts = np.asarray(inputs["out_verts"]).astype(np.int64)

    # ---- host-side packing ----
    shared = {}
    for l in range(1, L):
        s, e = int(offsets[l - 1]), int(offsets[l])
        At = np.zeros((l * PAD, PAD), np.float32)  # [src_padded, tgt]
        rows = (edge_src[s:e] // PER) * PAD + (edge_src[s:e] % PER)
        np.add.at(At, (rows, edge_dst[s:e]), w_edge[s:e])
        At *= A_SCALE
        A_hi = At.astype(float8_e4m3)
        A_lo = (At - A_hi.astype(np.float32)).astype(float8_e4m3)
        # [4l*128, 512] -> [2l kp, 2 kt, 128, 512] -> [128, 2l, 2, 512]
        def kp_form(a8):
            return np.ascontiguousarray(
                a8.reshape(2 * l, 2, 128, PAD).transpose(2, 0, 1, 3)
            )
        hi = kp_form(A_hi)
        if l in DR3_LAYERS:
            lo = kp_form(A_lo)
            # [128, 2l, 2(hilo), 2, 512]
            shared[f"A3_{l}"] = np.ascontiguousarray(
                np.stack([hi, lo], axis=2)
            )
        else:
            shared[f"A2_{l}"] = hi

    winT = np.zeros((K_IN, PAD), np.float32)
    winT[:IN_DIM, :PER] = W_in.T
    winT_re = np.ascontiguousarray(
        _pack_ptiles(winT, 7).reshape(128, 7, 4, 128).transpose(2, 0, 1, 3)
    ).astype(bfloat16)

    binP = np.zeros((PAD,), np.float32)
    binP[:PER] = b_in
    binP_re = np.ascontiguousarray(binP.reshape(4, 128).T)

    woutT = np.zeros((NT * 128, OUT_DIM), np.float32)
    pad_idx = (out_verts // PER) * PAD + (out_verts % PER)
    woutT[pad_idx, :] = W_out.T
    used_tiles = tuple(sorted(set(int(t) for t in pad_idx // 128)))
    woutT_re = np.ascontiguousarray(
        _pack_ptiles(woutT, NT)[:, list(used_tiles), :]
    ).astype(bfloat16)

    boutP = np.ascontiguousarray(b_out.reshape(OUT_DIM, 1))

    shared.update({
        "W_inT": winT_re,
        "b_inP": binP_re,
        "W_outT": woutT_re,
        "b_outP": boutP,
    })
    in_maps = []
    for c in range(NC):
        xT = np.zeros((K_IN, BL), np.float32)
        xT[:IN_DIM, :] = x[c * BL:(c + 1) * BL, :].T
        in_maps.append({"xT": _pack_ptiles(xT, 7).astype(bfloat16), **shared})

    from concourse.bass_utils import run_bass_kernel_spmd

    global _LAST_IN_MAPS, _PROG, _PROG_KEY
    _LAST_IN_MAPS = in_maps
    if _PROG is None or _PROG_KEY != used_tiles:
        _PROG = _build_program(used_tiles)
        _PROG_KEY = used_tiles
    res = run_bass_kernel_spmd(_PROG, in_maps, list(range(NC)))
    out = np.concatenate(
        [np.asarray(res.results[c]["out"], np.float32).T for c in range(NC)], axis=0
    )
    return np.ascontiguousarray(out)
